# revision 1
# baseline (speedup 1.0000x reference)
"""Multi-head attention (B=1, S=4096, D=1024, H=16) on 8 trn2 NeuronCores.

v4: head-parallel TP (2 heads/core), fp16 dataflow, exp-roofline schedule.
  - Exp on the scalar engine (33.5M elem/core, ~1.1 us per [128,1024]) is
    the roofline; the schedule keeps it fed continuously:
    * scores pipeline flows ACROSS q-block boundaries (lookahead wraps),
    * attnV runs at lag 2 behind exp, so drains never stall the Act feed,
    * all q/k/v projections live in phase 2 under qb0's attention.
  - Normalization is folded into the output projection: attnV rowsums
    (ones-column #65) are transposed to per-partition scalars, one
    reciprocal_approx_fast over [128,8], and the per-head outproj psums
    are combined with tensor_scalar ops - no broadcast matmuls, no
    per-element normalize pass.
  - Inputs stream as per-dc [128,1024] fp16 DMAs staged 2+ passes ahead.
  - Output [S, D] fp16 partials; host sums in f32 and adds bo.
"""
import numpy as np

try:
    import concourse.bass as bass  # noqa: F401
except ImportError:  # grading env fallback
    import sys
    for p in ("/opt/trn_rl_repo", "/opt/pypackages"):
        if p not in sys.path:
            sys.path.insert(0, p)

S = 4096
D_MODEL = 1024
N_CORES = 8
DH = 128              # head dims owned by one core (2 heads x 64)
D_K = 64
SB = 512              # s-block granularity (projection passes, q-blocks)
NSB = S // SB         # 8
QB = 512
NQB = 8
KC = 128              # k-chunk (attnV contraction tile)
NKC = S // KC         # 32
KPP = NKC // NSB      # kc chunks per kv pass (4)
SCALE = float(D_K) ** -0.5
DSCALE = 0.125        # drain scale; cancels via the rowsum reciprocal

TRACE = False          # set by test harness for NTFF profiling
_CACHE = {}


def _build_nc():
    import concourse.bacc as bacc
    import concourse.tile as tile
    from concourse import mybir
    from concourse.masks import make_identity

    f32 = mybir.dt.float32
    f16 = mybir.dt.float16
    Exp = mybir.ActivationFunctionType.Exp
    MULT = mybir.AluOpType.mult
    ADD = mybir.AluOpType.add

    nc = bacc.Bacc("TRN2", target_bir_lowering=False, debug=False,
                   num_devices=N_CORES)

    xq = nc.dram_tensor("xq", [D_MODEL, S], f16, kind="ExternalInput")
    xk = nc.dram_tensor("xk", [D_MODEL, S], f16, kind="ExternalInput")
    xv = nc.dram_tensor("xv", [D_MODEL, S], f16, kind="ExternalInput")
    wq = nc.dram_tensor("wq", [D_MODEL, DH], f16, kind="ExternalInput")
    wk = nc.dram_tensor("wk", [D_MODEL, DH], f16, kind="ExternalInput")
    wv = nc.dram_tensor("wv", [D_MODEL, DH], f16, kind="ExternalInput")
    wo = nc.dram_tensor("wo", [DH, D_MODEL], f16, kind="ExternalInput")
    bqv = nc.dram_tensor("bq", [DH, 1], f32, kind="ExternalInput")
    bkv = nc.dram_tensor("bk", [DH, 1], f32, kind="ExternalInput")
    bvv = nc.dram_tensor("bv", [DH, 1], f32, kind="ExternalInput")
    out = nc.dram_tensor("out", [S, D_MODEL], f16, kind="ExternalOutput")

    with tile.TileContext(nc) as tc:
        with (
            tc.tile_pool(name="big", bufs=1) as big,
            tc.tile_pool(name="xin", bufs=48) as xin,
            tc.tile_pool(name="at", bufs=5) as atp,
            tc.tile_pool(name="vt", bufs=2) as vtp,
            tc.tile_pool(name="cq", bufs=2) as cqp,
            tc.tile_pool(name="outs", bufs=3) as outs,
            tc.tile_pool(name="small", bufs=8) as small,
            tc.tile_pool(name="ps_s", bufs=2, space="PSUM") as ps_s,
            tc.tile_pool(name="ps_o", bufs=1, space="PSUM") as ps_o,
        ):
            # ---- constants -------------------------------------------------
            ident_f = big.tile([128, 128], f32, tag="ident_f")
            nc.vector.memset(ident_f, 0.0)
            make_identity(nc, ident_f, nomemset=True)
            ident = big.tile([128, 128], f16, tag="ident")
            nc.vector.tensor_copy(ident, ident_f)
            ident1 = big.tile([1, 1], f32, tag="ident1")
            nc.vector.memset(ident1, 1.0)

            # ---- weights / biases -----------------------------------------
            def w_tile(name, dram):
                t = big.tile([128, NSB, 128], f16, tag=name)
                nc.sync.dma_start(
                    out=t, in_=dram.rearrange("(c p) m -> p c m", p=128))
                return t

            wq_sb = w_tile("wq", wq)
            wk_sb = w_tile("wk", wk)
            wv_sb = w_tile("wv", wv)
            wo_sb = big.tile([128, D_MODEL], f16, tag="wo")
            nc.sync.dma_start(out=wo_sb, in_=wo[:, :])

            def b_tile(name, dram):
                t = big.tile([128, 1], f32, tag=name)
                nc.sync.dma_start(out=t, in_=dram[:, :])
                return t

            bq_sb = b_tile("bq", bqv)
            bk_sb = b_tile("bk", bkv)
            bv_sb = b_tile("bv", bvv)

            # ---- persistent activations -----------------------------------
            qt = big.tile([128, S], f16, tag="qt")    # [dh(2 heads), s]
            kt = big.tile([128, S], f16, tag="kt")
            vnat = big.tile([128, NKC, 2, 65], f16, tag="vnat")
            nc.vector.memset(vnat, 1.0)   # ones column rides at [.., 64]

            # ---- input staging: per-dc [128,1024] chunks ------------------
            xts = {"q": {}, "k": {}, "v": {}}
            xdr = {"q": xq, "k": xk, "v": xv}

            def stage_pair(nm, pp):
                ts = []
                s0 = pp * 2 * SB
                for dc in range(NSB):
                    t = xin.tile([128, 2 * SB], f16, tag="xt")
                    nc.sync.dma_start(
                        out=t,
                        in_=xdr[nm][dc * 128:(dc + 1) * 128, s0:s0 + 2 * SB])
                    ts.append(t)
                xts[nm][pp] = ts

            def proj_pass(dst_fn, w_sb, nm, sb):
                ts = xts[nm][sb // 2]
                off = (sb % 2) * SB
                psm = ps_s.tile([128, SB], f32, tag="s")
                for dc in range(NSB):
                    nc.tensor.matmul(psm, w_sb[:, dc, :],
                                     ts[dc][:, off:off + SB],
                                     start=dc == 0, stop=dc == NSB - 1)
                dst_fn(psm, sb)

            def qk_dst(dst, bias):
                def f(psm, sb):
                    nc.vector.tensor_scalar_add(
                        dst[:, sb * SB:(sb + 1) * SB], psm, bias)
                return f

            def v_dst(psm, sb):
                vt = vtp.tile([128, SB], f16, tag="vt")
                nc.vector.tensor_scalar_add(vt, psm, bv_sb)
                for i in range(KPP):
                    kc = sb * KPP + i
                    pt = ps_s.tile([128, 128], f16, tag="s")
                    nc.tensor.transpose(pt, vt[:, i * 128:(i + 1) * 128],
                                        ident)
                    nc.vector.tensor_copy(vnat[:, kc, 0, 0:64], pt[:, 0:64])
                    nc.vector.tensor_copy(vnat[:, kc, 1, 0:64], pt[:, 64:128])

            # ---- attention ------------------------------------------------
            sc_tiles = {}
            at_tiles = {}

            def scores(kc, qsl):
                sp = ps_s.tile([128, 2 * QB], f32, tag="s")
                ksl = slice(kc * KC, (kc + 1) * KC)
                nc.tensor.matmul(sp[:, 0:QB], kt[0:64, ksl],
                                 qt[0:64, qsl], start=True, stop=True)
                nc.tensor.matmul(sp[:, QB:2 * QB], kt[64:128, ksl],
                                 qt[64:128, qsl], start=True, stop=True)
                sc_tiles[(kc, qsl.start)] = sp

            def exp_part(kc, qsl, ahead):
                at = atp.tile([128, 2 * QB], f16, tag="at")
                nc.scalar.activation(at, sc_tiles.pop((kc, qsl.start)), Exp,
                                     scale=SCALE)
                at_tiles[(kc, qsl.start)] = at
                if ahead is not None:
                    scores(*ahead)

            def mk_po():
                po0a = ps_o.tile([65, QB], f32, tag="o0a")
                po0b = ps_o.tile([65, QB], f32, tag="o0b")
                po1a = ps_o.tile([65, QB], f32, tag="o1a")
                po1b = ps_o.tile([65, QB], f32, tag="o1b")
                return po0a, po0b, po1a, po1b

            def attnv_part(kc, qsl, pos):
                po0a, po0b, po1a, po1b = pos
                at = at_tiles.pop((kc, qsl.start))
                st, sp_ = kc == 0, kc == NKC - 1
                nc.tensor.matmul(po0a, vnat[0:64, kc, 0, :],
                                 at[0:64, 0:QB], start=st, stop=sp_)
                nc.tensor.matmul(po1b, vnat[64:128, kc, 1, :],
                                 at[64:128, QB:2 * QB], start=st, stop=sp_)
                nc.tensor.matmul(po0b, vnat[64:128, kc, 0, :],
                                 at[64:128, 0:QB], start=st, stop=sp_)
                nc.tensor.matmul(po1a, vnat[0:64, kc, 1, :],
                                 at[0:64, QB:2 * QB], start=st, stop=sp_)

            def normalize(pos):
                """Drain po -> c2 (fp16, x0.125) and per-q reciprocal
                rowsums rr [128, 8] (col h*4+i for s-chunk i)."""
                po0a, po0b, po1a, po1b = pos
                c2 = cqp.tile([128, QB], f16, tag="c2")
                rr = small.tile([128, 8], f32, tag="rr")
                rss = []
                for h, poa, pob in ((0, po0a, po0b), (1, po1a, po1b)):
                    rs = small.tile([1, QB], f32, tag="rs")
                    nc.vector.tensor_scalar_mul(rs, poa[64:65, :], DSCALE)
                    nc.vector.scalar_tensor_tensor(
                        rs, pob[64:65, :], DSCALE, rs, MULT, ADD)
                    rss.append(rs)
                    hsl = slice(h * 64, (h + 1) * 64)
                    nc.vector.tensor_scalar_mul(c2[hsl, :], poa[0:64, :],
                                                DSCALE)
                    nc.vector.scalar_tensor_tensor(
                        c2[hsl, :], pob[0:64, :], DSCALE, c2[hsl, :],
                        MULT, ADD)
                rsT = ps_o.tile([128, 8], f32, tag="o0a")
                for h in range(2):
                    for i in range(4):
                        nc.tensor.transpose(
                            rsT[:, h * 4 + i:h * 4 + i + 1],
                            rss[h][:, i * 128:(i + 1) * 128], ident1)
                rT = small.tile([128, 8], f32, tag="rT")
                nc.vector.tensor_copy(rT, rsT)
                nc.vector.reciprocal_approx_fast(rr, rT)
                return c2, rr

            def outproj_unit(c2, rr, qb, sc, nb):
                csl = slice(sc * 128, (sc + 1) * 128)
                nsl = slice(nb * SB, (nb + 1) * SB)
                pf = ps_s.tile([128, 2, QB], f32, tag="s")
                nc.tensor.matmul(pf[:, 0, :], c2[0:64, csl],
                                 wo_sb[0:64, nsl], start=True, stop=True)
                nc.tensor.matmul(pf[:, 1, :], c2[64:128, csl],
                                 wo_sb[64:128, nsl], start=True, stop=True)
                ob = outs.tile([128, QB], f16, tag="ob")
                nc.vector.tensor_scalar_mul(ob, pf[:, 0, :], rr[:, sc:sc + 1])
                nc.vector.scalar_tensor_tensor(
                    ob, pf[:, 1, :], rr[:, 4 + sc:5 + sc], ob, MULT, ADD)
                s0 = qb * QB + sc * 128
                nc.sync.dma_start(out=out[s0:s0 + 128, nsl], in_=ob)

            # ---- phase 1: head staging + q(qb0), q(qb1) -------------------
            stage_pair("q", 0)
            stage_pair("k", 0)
            stage_pair("v", 0)
            proj_pass(qk_dst(qt, bq_sb), wq_sb, "q", 0)
            proj_pass(qk_dst(qt, bq_sb), wq_sb, "q", 1)

            # ---- phase 2: software-pipelined projections + qb0 attention --
            # kproj runs one pass ahead so the 2-deep scores pipeline flows
            # through every pass boundary without PE stalls.
            qsl0 = slice(0, QB)
            qsl1 = slice(QB, 2 * QB)
            pos0 = mk_po()
            proj_pass(qk_dst(kt, bk_sb), wk_sb, "k", 0)
            scores(0, qsl0)
            scores(1, qsl0)
            for p in range(NSB):
                if p % 2 == 0 and p // 2 + 1 < 4:
                    stage_pair("k", p // 2 + 1)
                    stage_pair("v", p // 2 + 1)
                    stage_pair("q", p // 2 + 1)
                if 1 <= p < 7:
                    proj_pass(qk_dst(qt, bq_sb), wq_sb, "q", p + 1)
                proj_pass(v_dst, wv_sb, "v", p)
                if p + 1 < NSB:
                    proj_pass(qk_dst(kt, bk_sb), wk_sb, "k", p + 1)
                lo = p * KPP
                for kc in range(lo, lo + KPP):
                    if kc + 2 < NKC:
                        ahead = (kc + 2, qsl0)
                    else:
                        ahead = (kc + 2 - NKC, qsl1)
                    exp_part(kc, qsl0, ahead)
                    if kc >= 2:
                        attnv_part(kc - 2, qsl0, pos0)
            attnv_part(NKC - 2, qsl0, pos0)
            attnv_part(NKC - 1, qsl0, pos0)

            # ---- phase 3: qb1..7, pipeline flows across boundaries --------
            prev = pos0
            cr = {}
            for qb in range(1, NQB):
                qsl = slice(qb * QB, (qb + 1) * QB)
                nqsl = slice((qb + 1) * QB, (qb + 2) * QB)
                cr[qb - 1] = normalize(prev)
                pos = mk_po()
                for kc in range(NKC):
                    if kc + 2 < NKC:
                        ahead = (kc + 2, qsl)
                    elif qb < NQB - 1:
                        ahead = (kc + 2 - NKC, nqsl)
                    else:
                        ahead = None
                    exp_part(kc, qsl, ahead)
                    if kc >= 2:
                        attnv_part(kc - 2, qsl, pos)
                    if kc >= 8 and kc % 2 == 0 and kc < 8 + 16:
                        u = (kc - 8) // 2
                        c2p, rrp_ = cr[qb - 1]
                        outproj_unit(c2p, rrp_, qb - 1, u // 2, u % 2)
                attnv_part(NKC - 2, qsl, pos)
                attnv_part(NKC - 1, qsl, pos)
                prev = pos
            cr[NQB - 1] = normalize(prev)
            for u in range(8):
                c2p, rrp_ = cr[NQB - 1]
                outproj_unit(c2p, rrp_, NQB - 1, u // 2, u % 2)

    nc.compile()
    return nc


def kernel(query, key, value, Wq, bq, Wk, bk, Wv, bv, Wo, bo):
    from concourse.bass_utils import run_bass_kernel_spmd

    nc = _CACHE.get("nc")
    if nc is None:
        nc = _CACHE["nc"] = _build_nc()

    f32, f16 = np.float32, np.float16
    qT = np.ascontiguousarray(np.asarray(query, f32)[0].T.astype(f16))
    kT = np.ascontiguousarray(np.asarray(key, f32)[0].T.astype(f16))
    vT = np.ascontiguousarray(np.asarray(value, f32)[0].T.astype(f16))
    Wq = np.asarray(Wq, f32); Wk = np.asarray(Wk, f32)
    Wv = np.asarray(Wv, f32); Wo = np.asarray(Wo, f32)
    bq = np.asarray(bq, f32); bk = np.asarray(bk, f32)
    bv = np.asarray(bv, f32); bo = np.asarray(bo, f32)

    in_maps = []
    for cid in range(N_CORES):
        cs = slice(cid * DH, (cid + 1) * DH)
        in_maps.append({
            "xq": qT, "xk": kT, "xv": vT,
            "wq": np.ascontiguousarray(Wq[cs, :].T.astype(f16)),
            "wk": np.ascontiguousarray(Wk[cs, :].T.astype(f16)),
            "wv": np.ascontiguousarray(Wv[cs, :].T.astype(f16)),
            "wo": np.ascontiguousarray(Wo[:, cs].T.astype(f16)),
            "bq": bq[cs].reshape(DH, 1).copy(),
            "bk": bk[cs].reshape(DH, 1).copy(),
            "bv": bv[cs].reshape(DH, 1).copy(),
        })

    res = run_bass_kernel_spmd(nc, in_maps, core_ids=list(range(N_CORES)),
                               trace=TRACE)
    _CACHE["last_results"] = res
    acc = res.results[0]["out"].astype(np.float32)
    for cid in range(1, N_CORES):
        acc += res.results[cid]["out"]
    return (acc + bo).reshape(1, S, D_MODEL)

